# revision 1
# baseline (speedup 1.0000x reference)
"""Corr1d kernel for Trainium2 (Bass/Tile), self-contained.

Math: for x, y of shape [B=8, C=128, H=96, W=320] (fp32), MAX_DISP=10,
the reference computes, per (b, h, w):

    out = 1/(21*128) * sum_c [ x[c,w] * S_y[c,w] + y[c,w] * S_x[c,w] ]
      S_y[c,w] = sum_{d=1..10} y[c,w-d]   (zero padded)
      S_x[c,w] = sum_{d=0..10} x[c,w+d]   (zero padded)

Mapping: data-parallel over B across 8 cores (1 batch per core). C=128 on
SBUF partitions. Rows of W are laid out with 16-element zero pads on each
side (row stride 352) so shifted reads never cross row boundaries. The
sliding-window sums are computed with a single tensor_tensor_scan per
tensor:  state = (v[t] + state) - v[t - win]  which keeps the running
state equal to the causal window sum (bounded, no cancellation). Products
run on DVE in fp16 (2x mode), the channel reduction is a ones-matmul on
the PE accumulating both products into PSUM, and ACT applies the 1/2688
scale on the PSUM->SBUF copy.
"""

import numpy as np

import concourse.bacc as bacc
import concourse.bass as bass  # noqa: F401  (AP types re-exported)
import concourse.mybir as mybir
from concourse.tile import TileContext
from concourse.bass_utils import run_bass_kernel_spmd

B, C, H, W = 8, 128, 96, 320
MAX_DISP = 10
N_CORES = 8

HC = 16                 # H rows per chunk
NCHUNK = H // HC        # 6
PAD = 16                # per-row left/right zero pad
RS = PAD + W + PAD      # 352 row stride in SBUF
G = 16                  # leading guard columns (zeros)
NP = HC * RS            # 5632 scanned columns per chunk (11 * 512)
F = G + NP              # X/Y buffer width
FA = F + 16             # A/B buffer width (py reads A at +10)
SCALE = 1.0 / ((2 * MAX_DISP + 1) * C)

# Compute dtype on chip: float16 (DVE 2x products, half DMA traffic).
# Flip to float32 for an exactness A/B check (slower).
COMPUTE_DT = mybir.dt.float16

_CACHE = {}


def _build(compute_dt):
    fdt = compute_dt
    # Bacc (not plain Bass): its finalize() runs generate_event_semaphores,
    # which splits multi-wait sync conditions into CTRL_ES instructions —
    # DVE instruction formats only have one embedded wait slot.
    nc = bacc.Bacc("TRN2", target_bir_lowering=False)
    x = nc.dram_tensor("x", [C, H, W], fdt, kind="ExternalInput")
    y = nc.dram_tensor("y", [C, H, W], fdt, kind="ExternalInput")
    out = nc.dram_tensor("out", [H, W], mybir.dt.float32, kind="ExternalOutput")

    ADD = mybir.AluOpType.add
    SUB = mybir.AluOpType.subtract
    MUL = mybir.AluOpType.mult

    with TileContext(nc) as tc:
        with (
            tc.tile_pool(name="sb", bufs=1) as sb,
            tc.tile_pool(name="ps", bufs=6, space="PSUM") as ps,
            tc.tile_pool(name="ob", bufs=2) as ob,
        ):
            # Persistent buffers; pads stay zero after the initial memset
            # because DMA only ever writes the row-data column ranges.
            Xb = [sb.tile([128, F], fdt, name=f"xbuf{i}") for i in range(2)]
            Yb = [sb.tile([128, F], fdt, name=f"ybuf{i}") for i in range(2)]
            Aw = sb.tile([128, FA], fdt, name="awin")   # causal win-11 of x
            Bw = sb.tile([128, FA], fdt, name="bwin")   # causal win-10 of y (shifted +1)
            px = sb.tile([128, NP], fdt, name="px")
            py = sb.tile([128, NP], fdt, name="py")
            ones = sb.tile([128, 1], fdt, name="ones")

            nc.vector.memset(ones[:], 1.0)
            for t in (*Xb, *Yb, Aw, Bw):
                nc.vector.memset(t[:], 0.0)

            for i in range(NCHUNK):
                X = Xb[i % 2]
                Y = Yb[i % 2]
                h0 = i * HC

                # Load x/y rows into padded slots: row r data at
                # columns [G + r*RS + PAD, G + r*RS + PAD + W).
                dstx = X[:, G:F].rearrange("p (h w) -> p h w", h=HC)[
                    :, :, PAD : PAD + W
                ]
                nc.sync.dma_start(dstx, x[:, h0 : h0 + HC, :])
                dsty = Y[:, G:F].rearrange("p (h w) -> p h w", h=HC)[
                    :, :, PAD : PAD + W
                ]
                nc.sync.dma_start(dsty, y[:, h0 : h0 + HC, :])

                # Fences: the scan ISA struct (S2S2D2_STT, 64B) has no room
                # for multiple embedded sync waits, so hand the cross-engine
                # DMA waits to plain TT ops. Each fence reads a DMA-written
                # column and writes an (always zero) guard column that the
                # scan's data1 AP covers — a true RAW dep on the same engine,
                # so the scans need no waits of their own. Column 0 is a
                # permanent zero, so data * 0 keeps the guard at zero.
                nc.vector.tensor_tensor(
                    X[:, 5:6], X[:, G + PAD : G + PAD + 1], X[:, 0:1], MUL
                )
                nc.vector.tensor_tensor(
                    Y[:, 6:7], Y[:, G + PAD : G + PAD + 1], Y[:, 0:1], MUL
                )

                # A[t] = sum_{v=t-10..t} X[v]  via state=(X[t]+state)-X[t-11]
                nc.vector.tensor_tensor_scan(
                    Aw[:, G:F], X[:, G:F], X[:, G - 11 : F - 11], 0.0, ADD, SUB
                )
                # B[t+1] = sum_{v=t-9..t} Y[v] via state=(Y[t]+state)-Y[t-10]
                # (output shifted +1 so S_y[q] lands at column q, keeping the
                #  product reads 4B-aligned for the DVE 2x mode)
                nc.vector.tensor_tensor_scan(
                    Bw[:, G + 1 : F + 1], Y[:, G:F], Y[:, G - 10 : F - 10], 0.0, ADD, SUB
                )

                # px[q] = x[q] * S_y[q],  py[q] = y[q] * S_x[q] = y[q]*A[q+10]
                nc.vector.tensor_tensor(px[:, :], X[:, G:F], Bw[:, G:F], MUL)
                nc.vector.tensor_tensor(py[:, :], Y[:, G:F], Aw[:, G + 10 : F + 10], MUL)

                # Channel reduction: psum[0, q] = sum_c (px + py), then scale.
                outsb = ob.tile([1, NP], mybir.dt.float32, tag="outsb")
                for s in range(NP // 512):
                    pt = ps.tile([1, 512], mybir.dt.float32, tag="ps")
                    sl = slice(512 * s, 512 * (s + 1))
                    nc.tensor.matmul(pt[:], ones[:], px[:, sl], start=True, stop=False)
                    nc.tensor.matmul(pt[:], ones[:], py[:, sl], start=False, stop=True)
                    nc.scalar.mul(outsb[:, sl], pt[:], SCALE)

                # Extract the valid W columns of each row.
                src = outsb[:, :].rearrange("p (h w) -> p h w", h=HC)[
                    :, :, PAD : PAD + W
                ]
                nc.sync.dma_start(out[h0 : h0 + HC, :], src)

    # Bacc.finalize() runs the compile pipeline (register allocation,
    # generate_event_semaphores wait-splitting). The axon/PJRT run path
    # takes a prebuilt module and never calls it, so do it here.
    nc.finalize()
    return nc


def _get_nc():
    key = ("nc", str(COMPUTE_DT))
    if key not in _CACHE:
        _CACHE[key] = _build(COMPUTE_DT)
    return _CACHE[key]


def _np_dt():
    return np.float16 if COMPUTE_DT == mybir.dt.float16 else np.float32


def run(inputs, trace=False, trace_cores=None):
    """Run on hardware; returns (out [B,H,W] fp32, BassKernelResults)."""
    x = np.asarray(inputs["x"], dtype=np.float32)
    y = np.asarray(inputs["y"], dtype=np.float32)
    assert x.shape == (B, C, H, W) and y.shape == (B, C, H, W)
    dt = _np_dt()
    xh = np.ascontiguousarray(x.astype(dt))
    yh = np.ascontiguousarray(y.astype(dt))
    in_maps = [{"x": xh[b], "y": yh[b]} for b in range(B)]
    nc = _get_nc()
    res = run_bass_kernel_spmd(
        nc,
        in_maps,
        core_ids=list(range(N_CORES)),
        trace=trace,
        trace_cores=trace_cores,
    )
    outs = np.stack([r["out"] for r in res.results], axis=0).astype(np.float32)
    return outs, res


def kernel(**inputs) -> np.ndarray:
    out, _ = run(inputs, trace=False)
    return out



# revision 3
# speedup vs baseline: 1.1908x; 1.1908x over previous
"""Corr1d kernel for Trainium2 (Bass/Tile), self-contained.

Math: for x, y of shape [B=8, C=128, H=96, W=320] (fp32), MAX_DISP=10,
the reference computes, per (b, h, w):

    out = 1/(21*128) * sum_c [ x[c,w] * S_y[c,w] + y[c,w] * S_x[c,w] ]
      S_y[c,w] = sum_{d=1..10} y[c,w-d]   (zero padded)
      S_x[c,w] = sum_{d=0..10} x[c,w+d]   (zero padded)

Mapping: data-parallel over B across 8 cores (1 batch per core). C=128 on
SBUF partitions. Rows of W are laid out with a 12-col left pad and 10-col
right pad (row stride 342) so shifted reads never cross row data. The
sliding-window sums are computed with a single tensor_tensor_scan per
tensor:  state = (v[t] + state) - v[t - win]  which keeps the running
state equal to the causal window sum (bounded, no cancellation). Products
run on DVE in fp16 (2x mode), the channel reduction is a ones-matmul on
the PE accumulating both products into PSUM, and ACT applies the 1/2688
scale on the PSUM->SBUF copy. Pads are zeroed once with small strided
memsets (not whole-buffer memsets).
"""

import numpy as np

import concourse.bacc as bacc
import concourse.bass as bass  # noqa: F401  (AP types re-exported)
import concourse.mybir as mybir
from concourse.tile import TileContext
from concourse.bass_utils import run_bass_kernel_spmd

B, C, H, W = 8, 128, 96, 320
MAX_DISP = 10
N_CORES = 8

HC = 16                 # H rows per chunk
NCHUNK = H // HC        # 6
DL = 12                 # per-row left zero pad (>= 11 incl prev right pad)
DR = 10                 # per-row right zero pad (>= 10)
RS = DL + W + DR        # 342 row stride in SBUF
G = 16                  # leading guard columns (zeros)
NP = HC * RS            # 5472 scanned columns per chunk
F = G + NP              # end of scanned region
FB = F + 16             # buffer width (slack for strided pad memset / +10 reads)
SCALE = 1.0 / ((2 * MAX_DISP + 1) * C)

COMPUTE_DT = mybir.dt.float16

_CACHE = {}


def _build(compute_dt):
    fdt = compute_dt
    # Bacc (not plain Bass): its finalize() runs generate_event_semaphores,
    # which splits multi-wait sync conditions into CTRL_ES instructions —
    # DVE instruction formats only have one embedded wait slot.
    nc = bacc.Bacc("TRN2", target_bir_lowering=False)
    x = nc.dram_tensor("x", [C, H, W], fdt, kind="ExternalInput")
    y = nc.dram_tensor("y", [C, H, W], fdt, kind="ExternalInput")
    out = nc.dram_tensor("out", [H, W], mybir.dt.float32, kind="ExternalOutput")

    ADD = mybir.AluOpType.add
    SUB = mybir.AluOpType.subtract
    MUL = mybir.AluOpType.mult

    with TileContext(nc) as tc:
        with (
            tc.tile_pool(name="sb", bufs=1) as sb,
            tc.tile_pool(name="ps", bufs=6, space="PSUM") as ps,
            tc.tile_pool(name="ob", bufs=2) as ob,
        ):
            # Persistent buffers; pads stay zero after the initial memsets
            # because DMA only ever writes the row-data column ranges.
            Xb = [sb.tile([128, FB], fdt, name=f"xbuf{i}") for i in range(2)]
            Yb = [sb.tile([128, FB], fdt, name=f"ybuf{i}") for i in range(2)]
            Aw = sb.tile([128, FB], fdt, name="awin")   # causal win-11 of x
            Bw = sb.tile([128, FB], fdt, name="bwin")   # causal win-10 of y (shifted +1)
            px = sb.tile([128, NP], fdt, name="px")
            py = sb.tile([128, NP], fdt, name="py")
            ones = sb.tile([128, 1], fdt, name="ones")

            nc.vector.memset(ones[:], 1.0)
            # Pad zeroing: head [0, G+DL), then the 22-col gap after each
            # row's data ([data_end, next_data_start)), incl. 12 slack cols
            # past F for the last row (buffer is FB wide).
            for t in (*Xb, *Yb):
                nc.vector.memset(t[:, 0 : G + DL], 0.0)
                de = G + DL + W  # first row's data end
                gaps = t[:, de : de + (HC - 1) * RS].rearrange(
                    "p (h w) -> p h w", h=HC - 1
                )[:, :, 0 : DL + DR]
                nc.vector.memset(gaps, 0.0)
                nc.vector.memset(t[:, de + (HC - 1) * RS : de + (HC - 1) * RS + DL + DR], 0.0)
            # Aw: py reads [G+10, F+10) but the scan writes only [G, F).
            nc.vector.memset(Aw[:, F : F + 10], 0.0)
            # Bw: px reads [G, F) but the scan writes only [G+1, F+1).
            nc.vector.memset(Bw[:, G : G + 1], 0.0)

            for i in range(NCHUNK):
                X = Xb[i % 2]
                Y = Yb[i % 2]
                h0 = i * HC

                # Load x/y rows into padded slots: row r data at
                # columns [G + r*RS + DL, G + r*RS + DL + W).
                dstx = X[:, G:F].rearrange("p (h w) -> p h w", h=HC)[
                    :, :, DL : DL + W
                ]
                nc.sync.dma_start(dstx, x[:, h0 : h0 + HC, :])
                dsty = Y[:, G:F].rearrange("p (h w) -> p h w", h=HC)[
                    :, :, DL : DL + W
                ]
                nc.sync.dma_start(dsty, y[:, h0 : h0 + HC, :])

                # Fences: the scan ISA struct (S2S2D2_STT, 64B) has no room
                # for multiple embedded sync waits, so hand the cross-engine
                # DMA waits to plain TT ops. Each fence reads a DMA-written
                # column and writes an (always zero) guard column that the
                # scan's data1 AP covers — a true RAW dep on the same engine,
                # so the scans need no waits of their own. Column 0 is a
                # permanent zero, so data * 0 keeps the guard at zero.
                nc.vector.tensor_tensor(
                    X[:, 3:4], X[:, G + DL : G + DL + 1], X[:, 0:1], MUL
                )
                nc.vector.tensor_tensor(
                    Y[:, 4:5], Y[:, G + DL : G + DL + 1], Y[:, 0:1], MUL
                )

                # A[t] = sum_{v=t-10..t} X[v]  via state=(X[t]+state)-X[t-11]
                nc.vector.tensor_tensor_scan(
                    Aw[:, G:F], X[:, G:F], X[:, G - 11 : F - 11], 0.0, ADD, SUB
                )
                # B[t+1] = sum_{v=t-9..t} Y[v] via state=(Y[t]+state)-Y[t-10]
                # (output shifted +1 so S_y[q] lands at column q, keeping the
                #  product reads 4B-aligned for the DVE 2x mode)
                nc.vector.tensor_tensor_scan(
                    Bw[:, G + 1 : F + 1], Y[:, G:F], Y[:, G - 10 : F - 10], 0.0, ADD, SUB
                )

                # px[q] = x[q] * S_y[q],  py[q] = y[q] * S_x[q] = y[q]*A[q+10]
                nc.vector.tensor_tensor(px[:, :], X[:, G:F], Bw[:, G:F], MUL)
                nc.vector.tensor_tensor(py[:, :], Y[:, G:F], Aw[:, G + 10 : F + 10], MUL)

                # Channel reduction: psum[0, q] = sum_c (px + py), then scale.
                outsb = ob.tile([1, NP], mybir.dt.float32, tag="outsb")
                nslice = (NP + 511) // 512
                for s in range(nslice):
                    lo = 512 * s
                    hi = min(512 * (s + 1), NP)
                    pt = ps.tile([1, hi - lo], mybir.dt.float32, tag="ps")
                    sl = slice(lo, hi)
                    nc.tensor.matmul(pt[:], ones[:], px[:, sl], start=True, stop=False)
                    nc.tensor.matmul(pt[:], ones[:], py[:, sl], start=False, stop=True)
                    nc.scalar.mul(outsb[:, sl], pt[:], SCALE)

                # Extract the valid W columns of each row.
                src = outsb[:, :].rearrange("p (h w) -> p h w", h=HC)[
                    :, :, DL : DL + W
                ]
                nc.sync.dma_start(out[h0 : h0 + HC, :], src)

    # Bacc.finalize() runs the compile pipeline (register allocation,
    # generate_event_semaphores wait-splitting). The axon/PJRT run path
    # takes a prebuilt module and never calls it, so do it here.
    nc.finalize()
    return nc


def _get_nc():
    key = ("nc", str(COMPUTE_DT))
    if key not in _CACHE:
        _CACHE[key] = _build(COMPUTE_DT)
    return _CACHE[key]


def _np_dt():
    return np.float16 if COMPUTE_DT == mybir.dt.float16 else np.float32


def run(inputs, trace=False, trace_cores=None):
    """Run on hardware; returns (out [B,H,W] fp32, BassKernelResults)."""
    x = np.asarray(inputs["x"], dtype=np.float32)
    y = np.asarray(inputs["y"], dtype=np.float32)
    assert x.shape == (B, C, H, W) and y.shape == (B, C, H, W)
    dt = _np_dt()
    xh = np.ascontiguousarray(x.astype(dt))
    yh = np.ascontiguousarray(y.astype(dt))
    in_maps = [{"x": xh[b], "y": yh[b]} for b in range(B)]
    nc = _get_nc()
    res = run_bass_kernel_spmd(
        nc,
        in_maps,
        core_ids=list(range(N_CORES)),
        trace=trace,
        trace_cores=trace_cores,
    )
    outs = np.stack([r["out"] for r in res.results], axis=0).astype(np.float32)
    return outs, res


def kernel(**inputs) -> np.ndarray:
    out, _ = run(inputs, trace=False)
    return out


# revision 4
# speedup vs baseline: 1.1921x; 1.0011x over previous
"""Corr1d kernel for Trainium2 (Bass/Tile), self-contained.

Math: for x, y of shape [B=8, C=128, H=96, W=320] (fp32), MAX_DISP=10,
the reference computes, per (b, h, w):

    out = 1/(21*128) * sum_c [ x[c,w] * S_y[c,w] + y[c,w] * S_x[c,w] ]
      S_y[c,w] = sum_{d=1..10} y[c,w-d]   (zero padded)
      S_x[c,w] = sum_{d=0..10} x[c,w+d]   (zero padded)

Mapping: data-parallel over B across 8 cores (1 batch per core). C=128 on
SBUF partitions. Rows of W are laid out with a 12-col left pad and 10-col
right pad (row stride 342) so shifted reads never cross row data. The
sliding-window sums are computed with a single tensor_tensor_scan per
tensor:  state = (v[t] + state) - v[t - win]  which keeps the running
state equal to the causal window sum (bounded, no cancellation). Products
run on DVE in fp16 (2x mode), the channel reduction is a ones-matmul on
the PE accumulating both products into PSUM, and ACT applies the 1/2688
scale on the PSUM->SBUF copy. Pads are zeroed once with small strided
memsets (not whole-buffer memsets).
"""

import numpy as np

import concourse.bacc as bacc
import concourse.bass as bass  # noqa: F401  (AP types re-exported)
import concourse.mybir as mybir
from concourse.tile import TileContext
from concourse.bass_utils import run_bass_kernel_spmd

B, C, H, W = 8, 128, 96, 320
MAX_DISP = 10
N_CORES = 8

HC = 16                 # H rows per chunk
NCHUNK = H // HC        # 6
DL = 12                 # per-row left zero pad (>= 11 incl prev right pad)
DR = 10                 # per-row right zero pad (>= 10)
RS = DL + W + DR        # 342 row stride in SBUF
G = 16                  # leading guard columns (zeros)
NP = HC * RS            # 5472 scanned columns per chunk
F = G + NP              # end of scanned region
FB = F + 16             # buffer width (slack for strided pad memset / +10 reads)
SCALE = 1.0 / ((2 * MAX_DISP + 1) * C)

COMPUTE_DT = mybir.dt.float16

_CACHE = {}


def _build(compute_dt):
    fdt = compute_dt
    # Bacc (not plain Bass): its finalize() runs generate_event_semaphores,
    # which splits multi-wait sync conditions into CTRL_ES instructions —
    # DVE instruction formats only have one embedded wait slot.
    nc = bacc.Bacc("TRN2", target_bir_lowering=False)
    x = nc.dram_tensor("x", [C, H, W], fdt, kind="ExternalInput")
    y = nc.dram_tensor("y", [C, H, W], fdt, kind="ExternalInput")
    out = nc.dram_tensor("out", [H, W], mybir.dt.float32, kind="ExternalOutput")

    ADD = mybir.AluOpType.add
    SUB = mybir.AluOpType.subtract
    MUL = mybir.AluOpType.mult

    with TileContext(nc) as tc:
        with (
            tc.tile_pool(name="sb", bufs=1) as sb,
            tc.tile_pool(name="ps", bufs=6, space="PSUM") as ps,
            tc.tile_pool(name="ob", bufs=2) as ob,
        ):
            # Persistent buffers; pads stay zero after the initial memsets
            # because DMA only ever writes the row-data column ranges.
            Xb = [sb.tile([128, FB], fdt, name=f"xbuf{i}") for i in range(2)]
            Yb = [sb.tile([128, FB], fdt, name=f"ybuf{i}") for i in range(2)]
            Aw = sb.tile([128, FB], fdt, name="awin")   # causal win-11 of x
            Bw = sb.tile([128, FB], fdt, name="bwin")   # causal win-10 of y (shifted +1)
            px = sb.tile([128, NP], fdt, name="px")
            py = sb.tile([128, NP], fdt, name="py")
            ones = sb.tile([128, 1], fdt, name="ones")

            nc.vector.memset(ones[:], 1.0)
            # Pad zeroing: head [0, G+DL), then the 22-col gap after each
            # row's data ([data_end, next_data_start)), incl. 12 slack cols
            # past F for the last row (buffer is FB wide).
            for t in (*Xb, *Yb):
                nc.vector.memset(t[:, 0 : G + DL], 0.0)
                de = G + DL + W  # first row's data end
                gaps = t[:, de : de + (HC - 1) * RS].rearrange(
                    "p (h w) -> p h w", h=HC - 1
                )[:, :, 0 : DL + DR]
                nc.vector.memset(gaps, 0.0)
                nc.vector.memset(t[:, de + (HC - 1) * RS : de + (HC - 1) * RS + DL + DR], 0.0)
            # Aw: py reads [G+10, F+10) but the scan writes only [G, F).
            nc.vector.memset(Aw[:, F : F + 10], 0.0)
            # Bw: px reads [G, F) but the scan writes only [G+1, F+1).
            nc.vector.memset(Bw[:, G : G + 1], 0.0)

            # First and last chunks are split in half so the first scan
            # starts after only 8 rows of X have landed, and the final
            # PE/ACT/DMA tail follows a half-size scan.
            sched = [(0, 8), (8, 8)]
            sched += [(16 * i, 16) for i in range(1, NCHUNK - 1)]
            sched += [(H - 16, 8), (H - 8, 8)]

            for i, (h0, hc) in enumerate(sched):
                X = Xb[i % 2]
                Y = Yb[i % 2]
                NPc = hc * RS
                Fc = G + NPc

                # Load x/y rows into padded slots: row r data at
                # columns [G + r*RS + DL, G + r*RS + DL + W).
                dstx = X[:, G:Fc].rearrange("p (h w) -> p h w", h=hc)[
                    :, :, DL : DL + W
                ]
                nc.sync.dma_start(dstx, x[:, h0 : h0 + hc, :])
                dsty = Y[:, G:Fc].rearrange("p (h w) -> p h w", h=hc)[
                    :, :, DL : DL + W
                ]
                nc.sync.dma_start(dsty, y[:, h0 : h0 + hc, :])

                # Fences: the scan ISA struct (S2S2D2_STT, 64B) has no room
                # for multiple embedded sync waits, so hand the cross-engine
                # DMA waits to plain TT ops. Each fence reads a DMA-written
                # column and writes an (always zero) guard column that the
                # scan's data1 AP covers — a true RAW dep on the same engine,
                # so the scans need no waits of their own. Column 0 is a
                # permanent zero, so data * 0 keeps the guard at zero.
                # The Y fence sits between the scans so the X scan does not
                # serialize behind the Y DMA.
                nc.vector.tensor_tensor(
                    X[:, 3:4], X[:, G + DL : G + DL + 1], X[:, 0:1], MUL
                )
                # A[t] = sum_{v=t-10..t} X[v]  via state=(X[t]+state)-X[t-11]
                nc.vector.tensor_tensor_scan(
                    Aw[:, G:Fc], X[:, G:Fc], X[:, G - 11 : Fc - 11], 0.0, ADD, SUB
                )
                nc.vector.tensor_tensor(
                    Y[:, 4:5], Y[:, G + DL : G + DL + 1], Y[:, 0:1], MUL
                )
                # B[t+1] = sum_{v=t-9..t} Y[v] via state=(Y[t]+state)-Y[t-10]
                # (output shifted +1 so S_y[q] lands at column q, keeping the
                #  product reads 4B-aligned for the DVE 2x mode)
                nc.vector.tensor_tensor_scan(
                    Bw[:, G + 1 : Fc + 1], Y[:, G:Fc], Y[:, G - 10 : Fc - 10], 0.0, ADD, SUB
                )

                # px[q] = x[q] * S_y[q],  py[q] = y[q] * S_x[q] = y[q]*A[q+10]
                # (For half chunks the trailing 10 columns of the Aw read are
                #  leftovers from the previous chunk's scan: finite garbage
                #  that only reaches pad columns, which are never DMA'd out.)
                nc.vector.tensor_tensor(px[:, 0:NPc], X[:, G:Fc], Bw[:, G:Fc], MUL)
                nc.vector.tensor_tensor(
                    py[:, 0:NPc], Y[:, G:Fc], Aw[:, G + 10 : Fc + 10], MUL
                )

                # Channel reduction: psum[0, q] = sum_c (px + py), then scale.
                outsb = ob.tile([1, NPc], mybir.dt.float32, tag="outsb")
                nslice = (NPc + 511) // 512
                for s in range(nslice):
                    lo = 512 * s
                    hi = min(512 * (s + 1), NPc)
                    pt = ps.tile([1, hi - lo], mybir.dt.float32, tag="ps")
                    sl = slice(lo, hi)
                    nc.tensor.matmul(pt[:], ones[:], px[:, sl], start=True, stop=False)
                    nc.tensor.matmul(pt[:], ones[:], py[:, sl], start=False, stop=True)
                    nc.scalar.mul(outsb[:, sl], pt[:], SCALE)

                # Extract the valid W columns of each row.
                src = outsb[:, :].rearrange("p (h w) -> p h w", h=hc)[
                    :, :, DL : DL + W
                ]
                nc.sync.dma_start(out[h0 : h0 + hc, :], src)

    # Bacc.finalize() runs the compile pipeline (register allocation,
    # generate_event_semaphores wait-splitting). The axon/PJRT run path
    # takes a prebuilt module and never calls it, so do it here.
    nc.finalize()
    return nc


def _get_nc():
    key = ("nc", str(COMPUTE_DT))
    if key not in _CACHE:
        _CACHE[key] = _build(COMPUTE_DT)
    return _CACHE[key]


def _np_dt():
    return np.float16 if COMPUTE_DT == mybir.dt.float16 else np.float32


def run(inputs, trace=False, trace_cores=None):
    """Run on hardware; returns (out [B,H,W] fp32, BassKernelResults)."""
    x = np.asarray(inputs["x"], dtype=np.float32)
    y = np.asarray(inputs["y"], dtype=np.float32)
    assert x.shape == (B, C, H, W) and y.shape == (B, C, H, W)
    dt = _np_dt()
    xh = np.ascontiguousarray(x.astype(dt))
    yh = np.ascontiguousarray(y.astype(dt))
    in_maps = [{"x": xh[b], "y": yh[b]} for b in range(B)]
    nc = _get_nc()
    res = run_bass_kernel_spmd(
        nc,
        in_maps,
        core_ids=list(range(N_CORES)),
        trace=trace,
        trace_cores=trace_cores,
    )
    outs = np.stack([r["out"] for r in res.results], axis=0).astype(np.float32)
    return outs, res


def kernel(**inputs) -> np.ndarray:
    out, _ = run(inputs, trace=False)
    return out
